# revision 1
# baseline (speedup 1.0000x reference)
"""Trainium2 Bass kernel: single-channel Conv2d.

  x: [32, 224, 224] f32, kernels: [64, 7, 7] f32
  out[b, k, i, j] = sum_{di,dj} x[b, i+di, j+dj] * kernels[k, di, dj]
  -> [32, 64, 218, 218]

Sharding: data-parallel over batch, 4 images per NeuronCore across 8 cores.

Per-core algorithm (bf16 matmuls, one stationary weight per PE half):
  - Host sends x as bf16 pre-interleaved per image-pair
    (xh[qp, row, img*224+j]) and a banded stationary matrix
        W[dr*8 + g, s*64 + k] = kernels[k, dr - s, g]   (dr 0..7, g 0..7,
    s 0..1; zero outside 0 <= dr-s <= 6, g <= 6) duplicated at PE rows
    0..63 and 64..127.  All 49 taps live in one 64-deep contraction, so
    every output-row-pair needs exactly ONE matmul.
  - An image-pair's rows are staged in SBUF as x2b[row, seg*464 + u]
    (u = img*224 + j; segments rows 0..127 / 96..223; 16-col zero pad).
  - ONE gather DMA builds pt[p = dr*8+g, u] = x2b[r0 + dr, seg_off+u+g]
    for TWO row-pairs at once (dr 0..15: rows r0..r0+15 feed pairs r0/2
    and r0/2+4): the 8 column shifts are overlapping stride-1 dims of the
    source AP, so no shift-expanded image copy is ever materialized.
  - Per row-pair, one matmul into ps[128 = (s,k), 448 = (img,j)]:
    pair A uses PE rows 0..63 (rhs/lhsT base 0), pair B PE rows 64..127.
  - VectorE+ScalarE evacuate PSUM into a 16-pair SBUF chunk [128, 16*448].
  - Each chunk is stored VERBATIM to DRAM (one DMA, 128 x 28.7KB fully
    contiguous descriptors) on the Pool (SWDGE) queue; the host undoes the
    (q, chunk, (s,k), (pl,img,j)) layout with a single numpy transpose.
    This keeps the SDMA engines byte-bound instead of descriptor-bound.
"""
import sys

sys.path.insert(0, "/opt/trn_rl_repo")

import numpy as np
import ml_dtypes

B, H, W = 32, 224, 224
KCH, KS = 64, 7
HO = WO = H - KS + 1  # 218
NCORES = 8
BLOC = B // NCORES    # 4 images per core
NPAIRS = HO // 2      # 109 output-row-pairs per image-pair

SEGW = 464            # x2b per-segment span (448 data + 16 zero pad)
X2F = 2 * SEGW        # 928
SEG1 = 96             # first row of segment 1 (rows 96..223)
NST = 448             # matmul stream length (2 imgs x 224)
DVE_COLS = 268        # PSUM evacuation split: VectorE cols, rest ScalarE
CH = 16               # row-pairs per output SBUF chunk
NCHUNK = 7            # chunks per image-pair (6*16 + 13 = 109)

_NC_CACHE = {}


def make_weight_band(kernels: np.ndarray) -> np.ndarray:
    """Stationary matrix [128, 128] (bf16): the 64-row band
    W[dr*8 + g, s*64 + k] = kernels[k, dr - s, g], duplicated at
    partitions 0..63 and 64..127 (PE quadrant rows 0 / 64)."""
    wb = np.zeros((64, 128), dtype=np.float32)
    for dr in range(8):
        for g in range(KS):
            for s in range(2):
                di = dr - s
                if 0 <= di < KS:
                    wb[dr * 8 + g, s * KCH: (s + 1) * KCH] = kernels[:, di, g]
    return np.vstack([wb, wb]).astype(ml_dtypes.bfloat16)


def _build_nc():
    import concourse.bacc as bacc
    import concourse.mybir as mybir
    import concourse.tile as tile
    from concourse.bass_types import AP

    F32 = mybir.dt.float32
    BF16 = mybir.dt.bfloat16

    nc = bacc.Bacc("TRN2", target_bir_lowering=False, debug=False,
                   num_devices=NCORES)
    # x pre-interleaved on host: [image-pair, row, img*224+j]
    x_d = nc.dram_tensor("x", [2, H, 2 * W], BF16, kind="ExternalInput").ap()
    wb_d = nc.dram_tensor("wband", [128, 128], BF16,
                          kind="ExternalInput").ap()
    # raw chunk dump (bf16; host untangles the layout and upcasts)
    out_d = nc.dram_tensor("out", [2, NCHUNK, 128, CH * NST], BF16,
                           kind="ExternalOutput").ap()

    with tile.TileContext(nc) as tc:
        with (
            tc.tile_pool(name="wpool", bufs=1) as wpool,
            tc.tile_pool(name="x2pool", bufs=2) as x2pool,
            tc.tile_pool(name="ptpool", bufs=6) as ptpool,
            tc.tile_pool(name="opool", bufs=3) as opool,
            tc.tile_pool(name="psum", bufs=4, space="PSUM") as psum,
        ):
            wbt = wpool.tile([128, 128], BF16)
            nc.sync.dma_start(out=wbt[:], in_=wb_d)

            for q in range(2):
                x2b = x2pool.tile([128, X2F], BF16, tag="x2b")
                # zero the 16-col pads (cols 448..463 / 912..927)
                nc.gpsimd.memset(x2b[:, 448:464], 0.0)
                nc.gpsimd.memset(x2b[:, 912:928], 0.0)
                for seg in range(2):
                    r_lo = 0 if seg == 0 else SEG1
                    nc.sync.dma_start(
                        out=x2b[0:128, seg * SEGW: seg * SEGW + 2 * W],
                        in_=x_d[q, r_lo: r_lo + 128, :],
                    )

                chunks = {}  # chunk_start -> [tile, npl, n_evacuated]

                def get_chunk(pr):
                    cs = (pr // CH) * CH
                    if cs not in chunks:
                        npl = min(CH, NPAIRS - cs)
                        chunks[cs] = [opool.tile([128, CH * NST], BF16,
                                                 tag="osb", name="chunk"),
                                      npl, 0]
                    return cs, chunks[cs]

                def evac_and_store(pr, ps, q):
                    cs, ent = get_chunk(pr)
                    chunk, npl = ent[0], ent[1]
                    pl = pr - cs
                    nc.vector.tensor_copy(
                        out=chunk[:, pl * NST: pl * NST + DVE_COLS],
                        in_=ps[:, 0:DVE_COLS])
                    nc.scalar.copy(
                        out=chunk[:, pl * NST + DVE_COLS: (pl + 1) * NST],
                        in_=ps[:, DVE_COLS:NST])
                    ent[2] += 1
                    if ent[2] == npl:
                        nc.gpsimd.dma_start(
                            out=out_d[q, cs // CH, :, 0: npl * NST],
                            in_=chunk[:, 0: npl * NST])

                # groups of 8 pairs; 4 gathers per group, 2 pairs each
                for t in range(14):
                    for u in range(4):
                        prA = 8 * t + u
                        prB = prA + 4
                        if prA >= NPAIRS:
                            break
                        has_b = prB < NPAIRS
                        r0 = 2 * prA
                        nr = 16 if has_b else 8
                        seg = 0 if r0 + nr - 1 <= 127 else 1
                        roff = r0 - SEG1 * seg
                        pt = ptpool.tile([128, NST], BF16, tag="pt")
                        src = AP(
                            tensor=x2b[:].tensor,
                            offset=x2b[:].offset + roff * X2F + seg * SEGW,
                            ap=((X2F, nr), (1, 8), (1, NST)),
                        )
                        dst = AP(
                            tensor=pt[:].tensor,
                            offset=pt[:].offset,
                            ap=((NST, nr * 8), (1, NST)),
                        )
                        g_eng = nc.sync if (t + u) % 2 == 0 else nc.gpsimd
                        g_eng.dma_start(out=dst, in_=src)

                        psA = psum.tile([128, NST], F32)
                        nc.tensor.matmul(
                            out=psA[:], lhsT=wbt[0:64, :],
                            rhs=pt[0:64, :], start=True, stop=True)
                        evac_and_store(prA, psA, q)
                        if has_b:
                            psB = psum.tile([128, NST], F32)
                            nc.tensor.matmul(
                                out=psB[:], lhsT=wbt[64:128, :],
                                rhs=pt[64:128, :], start=True, stop=True)
                            evac_and_store(prB, psB, q)
    nc.compile()
    return nc


def _get_nc():
    if "nc" not in _NC_CACHE:
        _NC_CACHE["nc"] = _build_nc()
    return _NC_CACHE["nc"]


def _prep_x(x: np.ndarray) -> np.ndarray:
    """[B, H, W] f32 -> per-core [2, H, 2W] bf16, images interleaved."""
    xb = x.astype(ml_dtypes.bfloat16)
    # [B,H,W] -> [B//2 pairs, 2, H, W] -> [pairs, H, 2, W] -> [pairs, H, 2W]
    xp = xb.reshape(B // 2, 2, H, W).transpose(0, 2, 1, 3).reshape(
        B // 2, H, 2 * W)
    return np.ascontiguousarray(xp)


def _untangle(arr: np.ndarray) -> np.ndarray:
    """Per-core raw chunk dump [2, NCHUNK, 128, CH*NST] bf16 ->
    [4, KCH, HO, WO] f32."""
    # [q, ci, s*64+k, pl*448 + img*224 + j]
    a = arr.reshape(2, NCHUNK, 2, KCH, CH, 2, W)      # q ci s k pl img j
    a = a.transpose(0, 5, 3, 1, 4, 2, 6)              # q img k ci pl s j
    a = a.reshape(BLOC, KCH, 2 * NCHUNK * CH, W)      # rows = 224
    return a[:, :, :HO, :WO].astype(np.float32)


def kernel(x: np.ndarray, kernels: np.ndarray) -> np.ndarray:
    from concourse.bass_utils import run_bass_kernel_spmd

    x = np.asarray(x, dtype=np.float32)
    kernels = np.asarray(kernels, dtype=np.float32)
    xp = _prep_x(x)  # [16, H, 448]
    wb = make_weight_band(kernels)
    nc = _get_nc()
    in_maps = [
        {"x": xp[c * 2: c * 2 + 2], "wband": wb}
        for c in range(NCORES)
    ]
    res = run_bass_kernel_spmd(nc, in_maps, core_ids=list(range(NCORES)))
    return np.ascontiguousarray(np.concatenate(
        [_untangle(res.results[c]["out"]) for c in range(NCORES)], axis=0))



# revision 6
# speedup vs baseline: 1.3820x; 1.3820x over previous
"""Trainium2 Bass kernel: single-channel Conv2d.

  x: [32, 224, 224] f32, kernels: [64, 7, 7] f32
  out[b, k, i, j] = sum_{di,dj} x[b, i+di, j+dj] * kernels[k, di, dj]
  -> [32, 64, 218, 218]

Sharding: data-parallel over batch, 4 images per NeuronCore across 8 cores.

Per-core algorithm (bf16 matmuls, 8-output-row groups):
  - Host sends x as bf16 pre-interleaved per image-pair plus a 4KB zero
    tail (flat [2*224*448 + 4096]), and a banded stationary matrix
        wb4[dr*8 + g, kq*128 + s*16 + kc] = kernels[kq*16+kc, dr-s, g]
    (dr 0..13, g 0..7, s 0..7; zero outside 0 <= dr-s <= 6, g <= 6).
  - x rows are staged per image-pair in two [128, 464] SBUF tiles
    (rows 0..127 / 96..223); each load is ONE fully-contiguous 118KB
    descriptor (the 16-col tail of each 464-elem row holds harmless
    over-read garbage that only ever feeds discarded output columns or
    zero-weight taps).
  - Per 8-output-row group (rows 8*grp..8*grp+7): one SWDGE gather DMA
    builds pt[p = dr*8+g, u] = xseg[r0 + dr, u + g] (112 partitions,
    14 rows x 8 col-shifts; the shifts are overlapping stride-1 dims of
    the source AP). This is 2.3x less gather traffic than per-row-pair
    gathers: 14 source rows serve 8 output rows at once.
  - 4 matmuls per group (one per channel-quarter kq), each
    [112-contraction, 128 out = (s,kc), 448 stream] into its own PSUM
    bank of a [128, 4, 512] f32 psum tile.
  - VectorE+ScalarE evacuate all 4 banks in ONE strided copy each
    (amortizes per-instruction fixed cost) into a 4-group SBUF chunk
    [128, 16, 448] bf16.
  - Chunks are stored verbatim to DRAM on the sync (HWDGE) queue, 128
    fully-contiguous 14336B descriptors each; the host undoes the
    (q, chunk, (s,kc), (gc,kq,img,j)) layout with one numpy transpose.
"""
import sys

sys.path.insert(0, "/opt/trn_rl_repo")

import numpy as np
import ml_dtypes

B, H, W = 32, 224, 224
KCH, KS = 64, 7
HO = WO = H - KS + 1  # 218
NCORES = 8
BLOC = B // NCORES    # 4 images per core

NST = 448             # matmul stream length (2 imgs x 224)
SEGW = 464            # staged x row pitch (448 data + 16 over-read pad)
SEG1 = 96             # first row of segment 1 (rows 96..223)
NGRP = 28             # 8-row groups per image-pair (28*8 = 224 rows)
GPC = 4               # groups per output chunk
NCHUNK = NGRP // GPC  # 7
DVE_COLS = 268        # PSUM evacuation split: VectorE cols, rest ScalarE
XIMG = H * SEGW       # 103936 elems per image-pair (rows host-padded to 464)

_NC_CACHE = {}


def make_weight_band(kernels: np.ndarray) -> np.ndarray:
    """Stationary matrix [128, 512] (bf16):
    wb4[dr*8 + g, kq*128 + s*16 + kc] = kernels[kq*16+kc, dr-s, g]."""
    wb = np.zeros((128, 512), dtype=np.float32)
    for s in range(8):
        for di in range(KS):
            dr = s + di
            for g in range(KS):
                for kq in range(4):
                    wb[dr * 8 + g, kq * 128 + s * 16: kq * 128 + s * 16 + 16] = \
                        kernels[kq * 16: kq * 16 + 16, di, g]
    return wb.astype(ml_dtypes.bfloat16)


def _build_nc():
    import concourse.bacc as bacc
    import concourse.mybir as mybir
    import concourse.tile as tile
    from concourse.bass_types import AP

    F32 = mybir.dt.float32
    BF16 = mybir.dt.bfloat16

    nc = bacc.Bacc("TRN2", target_bir_lowering=False, debug=False,
                   num_devices=NCORES)
    # x flat: per-core [2*224*464] bf16, images interleaved per pair,
    # rows padded to 464 elems on the host
    x_d = nc.dram_tensor("x", [2 * XIMG], BF16,
                         kind="ExternalInput").ap()
    wb_d = nc.dram_tensor("wband", [128, 512], BF16,
                          kind="ExternalInput").ap()
    # raw chunk dump (bf16; host untangles the layout and upcasts)
    out_d = nc.dram_tensor("out", [2, NCHUNK, 128, GPC * 4 * NST], BF16,
                           kind="ExternalOutput").ap()

    with tile.TileContext(nc) as tc:
        with (
            tc.tile_pool(name="wpool", bufs=1) as wpool,
            tc.tile_pool(name="xpool", bufs=2) as xpool,
            tc.tile_pool(name="ptpool", bufs=4) as ptpool,
            tc.tile_pool(name="opool", bufs=3) as opool,
            tc.tile_pool(name="psum", bufs=2, space="PSUM") as psum,
        ):
            wbt = wpool.tile([128, 512], BF16)
            nc.sync.dma_start(out=wbt[:], in_=wb_d)

            for q in range(2):
                # stage rows 0..127 (xa) and 96..223 (xb); each load is one
                # fully-contiguous descriptor (row pitch 464 over-reads 16
                # cols of the next row / the zero tail -- harmless).
                xa = xpool.tile([128, SEGW], BF16, tag="xa")
                xb = xpool.tile([128, SEGW], BF16, tag="xb")
                for seg, xt in ((0, xa), (1, xb)):
                    src = AP(
                        tensor=x_d.tensor,
                        offset=q * XIMG + (SEG1 * SEGW if seg else 0),
                        ap=((SEGW, 128), (1, SEGW)),
                    )
                    dst = AP(tensor=xt.tensor, offset=xt.offset,
                             ap=((SEGW, 128), (1, SEGW)))
                    nc.sync.dma_start(out=dst, in_=src)

                for ci in range(NCHUNK):
                    chunk = opool.tile([128, GPC * 4, NST], BF16, tag="osb",
                                       name="chunk")
                    for gc in range(GPC):
                        grp = ci * GPC + gc
                        r0 = 8 * grp
                        nrow = 14 if grp < NGRP - 1 else 8
                        seg = 0 if (r0 + nrow - 1) <= 127 else 1
                        xt = xa if seg == 0 else xb
                        roff = r0 - SEG1 * seg
                        np_ = nrow * 8  # gather partitions

                        pt = ptpool.tile([128, NST], BF16, tag="pt")
                        src = AP(
                            tensor=xt.tensor,
                            offset=xt.offset + roff * SEGW,
                            ap=((SEGW, nrow), (1, 8), (1, NST)),
                        )
                        dst = AP(
                            tensor=pt.tensor,
                            offset=pt.offset,
                            ap=((NST, np_), (1, NST)),
                        )
                        nc.gpsimd.dma_start(out=dst, in_=src)

                        ps = psum.tile([128, 4, 512], F32, name="ps")
                        for kq in range(4):
                            nc.tensor.matmul(
                                out=ps[:, kq, 0:NST],
                                lhsT=wbt[0:np_, kq * 128:(kq + 1) * 128],
                                rhs=pt[0:np_, :],
                                start=True, stop=True)
                        # evacuate all 4 banks in one strided copy per engine
                        nc.vector.tensor_copy(
                            out=chunk[:, 4 * gc:4 * gc + 4, 0:DVE_COLS],
                            in_=ps[:, :, 0:DVE_COLS])
                        nc.scalar.copy(
                            out=chunk[:, 4 * gc:4 * gc + 4, DVE_COLS:NST],
                            in_=ps[:, :, DVE_COLS:NST])
                    nc.sync.dma_start(
                        out=out_d[q, ci],
                        in_=chunk[:])
    nc.compile()
    return nc


def _get_nc():
    if "nc" not in _NC_CACHE:
        _NC_CACHE["nc"] = _build_nc()
    return _NC_CACHE["nc"]


def _prep_x(x: np.ndarray) -> np.ndarray:
    """[B, H, W] f32 -> per-core flat [2*XIMG] bf16 x NCORES, images
    interleaved per pair, rows padded to SEGW (=464) elems."""
    xb = x.astype(ml_dtypes.bfloat16)
    # [B,H,W] -> [B//2 pairs, 2, H, W] -> [pairs, H, 2, W] = [pairs, H, 2W]
    xp = xb.reshape(B // 2, 2, H, W).transpose(0, 2, 1, 3)
    buf = np.zeros((B // 2, H, SEGW), dtype=ml_dtypes.bfloat16)
    buf[:, :, :2 * W] = xp.reshape(B // 2, H, 2 * W)
    return buf.reshape(NCORES, 2 * XIMG)


def _untangle(arr: np.ndarray) -> np.ndarray:
    """Per-core raw chunk dump [2, NCHUNK, 128, GPC*4*NST] bf16 ->
    [4, KCH, HO, WO] f32."""
    # [q, ci, s*16+kc, ((gc*4 + kq)*2 + img)*224 + j]
    a = arr.reshape(2, NCHUNK, 8, 16, GPC, 4, 2, W)  # q ci s kc gc kq img j
    a = a.transpose(0, 6, 5, 3, 1, 4, 2, 7)          # q img kq kc ci gc s j
    a = a.reshape(BLOC, KCH, H, W)
    return a[:, :, :HO, :WO].astype(np.float32)


def kernel(x: np.ndarray, kernels: np.ndarray) -> np.ndarray:
    from concourse.bass_utils import run_bass_kernel_spmd

    x = np.asarray(x, dtype=np.float32)
    kernels = np.asarray(kernels, dtype=np.float32)
    xp = _prep_x(x)  # [NCORES, 2*XIMG+XPAD]
    wb = make_weight_band(kernels)
    nc = _get_nc()
    in_maps = [
        {"x": xp[c], "wband": wb}
        for c in range(NCORES)
    ]
    res = run_bass_kernel_spmd(nc, in_maps, core_ids=list(range(NCORES)))
    return np.ascontiguousarray(np.concatenate(
        [_untangle(res.results[c]["out"]) for c in range(NCORES)], axis=0))


# revision 8
# speedup vs baseline: 1.4375x; 1.0401x over previous
"""Trainium2 Bass kernel: single-channel Conv2d.

  x: [32, 224, 224] f32, kernels: [64, 7, 7] f32
  out[b, k, i, j] = sum_{di,dj} x[b, i+di, j+dj] * kernels[k, di, dj]
  -> [32, 64, 218, 218]

Sharding: data-parallel over batch, 4 images per NeuronCore across 8 cores.

Per-core algorithm (bf16 matmuls, 8-output-row groups, 2 groups per step):
  - Host sends x as bf16 pre-interleaved per image-pair with rows padded
    to 464 elems (flat [2*224*464]), and a banded stationary matrix
        wb[dr*7 + g, kq*128 + s*16 + kc] = kernels[kq*16+kc, dr-s, g]
    (dr 0..13, g 0..6, s 0..7; zero outside 0 <= dr-s <= 6).
  - x rows are staged per image-pair in two [128, 464] SBUF tiles
    (rows 0..127 / 96..223); each load is ONE fully-contiguous 118KB
    descriptor on the scalar (HWDGE) queue.
  - Per 2-group step: one SWDGE gather DMA builds
    pt2[p = dr*7+g, grp, u] = xseg[r0 + 8*grp + dr, u + g] for both
    groups (98 partitions x 2; the 7 col-shifts are overlapping
    stride-1 dims of the source AP).
  - 8 matmuls per step, kq-interleaved so consecutive matmul pairs
    share the stationary operand: (A,kq0)(B,kq0) -> psum tile1 banks
    0,1; (A,kq1)(B,kq1) -> banks 2,3; kq2/kq3 -> tile2. Each matmul is
    [98-contraction, 128 out = (s,kc), 448 stream].
  - Bank-split evacuation: VectorE copies banks 0-1 of each psum tile
    (full 448 cols), ScalarE banks 2-3 -- each can start mid-burst and
    the balanced split keeps both engines at the same occupancy.
  - Chunk [128, 8, 448] bf16 = one step; stored verbatim to DRAM on the
    sync (HWDGE) queue (128 contiguous 7168B descriptors); the host
    undoes the (q, step, (s,kc), (kq,grp,img,j)) layout with one numpy
    transpose.
"""
import sys

sys.path.insert(0, "/opt/trn_rl_repo")

import numpy as np
import ml_dtypes

B, H, W = 32, 224, 224
KCH, KS = 64, 7
HO = WO = H - KS + 1  # 218
NCORES = 8
BLOC = B // NCORES    # 4 images per core

NST = 448             # matmul stream length (2 imgs x 224)
SEGW = 464            # staged x row pitch (448 data + 16 pad)
SEG1 = 96             # first row of segment 1 (rows 96..223)
NSTEP = 14            # 2-group steps per image-pair (28 groups, 224 rows)
XIMG = H * SEGW       # 103936 elems per image-pair (rows host-padded)

_NC_CACHE = {}


def make_weight_band(kernels: np.ndarray) -> np.ndarray:
    """Stationary matrix [128, 512] (bf16):
    wb[dr*7 + g, kq*128 + s*16 + kc] = kernels[kq*16+kc, dr-s, g]."""
    wb = np.zeros((128, 512), dtype=np.float32)
    for s in range(8):
        for di in range(KS):
            dr = s + di
            for g in range(KS):
                for kq in range(4):
                    wb[dr * 7 + g, kq * 128 + s * 16: kq * 128 + s * 16 + 16] = \
                        kernels[kq * 16: kq * 16 + 16, di, g]
    return wb.astype(ml_dtypes.bfloat16)


def _build_nc():
    import concourse.bacc as bacc
    import concourse.mybir as mybir
    import concourse.tile as tile
    from concourse.bass_types import AP

    F32 = mybir.dt.float32
    BF16 = mybir.dt.bfloat16

    nc = bacc.Bacc("TRN2", target_bir_lowering=False, debug=False,
                   num_devices=NCORES)
    # x flat: per-core [2*224*464] bf16, images interleaved per pair,
    # rows padded to 464 elems on the host
    x_d = nc.dram_tensor("x", [2 * XIMG], BF16, kind="ExternalInput").ap()
    wb_d = nc.dram_tensor("wband", [128, 512], BF16,
                          kind="ExternalInput").ap()
    # raw chunk dump (bf16; host untangles the layout and upcasts)
    out_d = nc.dram_tensor("out", [2, NSTEP, 128, 8 * NST], BF16,
                           kind="ExternalOutput").ap()

    with tile.TileContext(nc) as tc:
        with (
            tc.tile_pool(name="wpool", bufs=1) as wpool,
            tc.tile_pool(name="xpool", bufs=2) as xpool,
            tc.tile_pool(name="ptpool", bufs=3) as ptpool,
            tc.tile_pool(name="opool", bufs=3) as opool,
            tc.tile_pool(name="psum", bufs=2, space="PSUM") as psum,
        ):
            wbt = wpool.tile([128, 512], BF16)
            nc.scalar.dma_start(out=wbt[:], in_=wb_d)

            xts = {}
            for q in range(2):
                # stage rows 0..127 (xa) and 96..223 (xb); each load is
                # one fully-contiguous descriptor on the scalar queue.
                xa = xpool.tile([128, SEGW], BF16, tag="xa", name="xa")
                xb = xpool.tile([128, SEGW], BF16, tag="xb", name="xb")
                xts[q] = (xa, xb)
                for seg, xt in ((0, xa), (1, xb)):
                    src = AP(
                        tensor=x_d.tensor,
                        offset=q * XIMG + (SEG1 * SEGW if seg else 0),
                        ap=((SEGW, 128), (1, SEGW)),
                    )
                    dst = AP(tensor=xt.tensor, offset=xt.offset,
                             ap=((SEGW, 128), (1, SEGW)))
                    nc.scalar.dma_start(out=dst, in_=src)

            for q in range(2):
                xa, xb = xts[q]
                for st in range(NSTEP):
                    r0 = 16 * st
                    seg = 0 if st <= 6 else 1
                    xt = xa if seg == 0 else xb
                    roff = r0 - SEG1 * seg
                    # gather both groups of the step in one SWDGE DMA;
                    # last step: group B has only 8 source rows.
                    nrA = 14
                    nrB = 14 if st < NSTEP - 1 else 8
                    pt2 = ptpool.tile([128, 2, NST], BF16, tag="pt")
                    for g, nr in ((0, nrA), (1, nrB)):
                        src = AP(
                            tensor=xt.tensor,
                            offset=xt.offset + (roff + 8 * g) * SEGW,
                            ap=((SEGW, nr), (1, KS), (1, NST)),
                        )
                        dst = AP(
                            tensor=pt2.tensor,
                            offset=pt2.offset + g * NST,
                            ap=((2 * NST, nr * KS), (1, NST)),
                        )
                        nc.gpsimd.dma_start(out=dst, in_=src)

                    chunk = opool.tile([128, 8, NST], BF16, tag="osb",
                                       name="chunk")
                    for half in range(2):  # kq pairs (0,1) and (2,3)
                        ps = psum.tile([128, 4, 512], F32, name="ps")
                        for kqh in range(2):
                            kq = 2 * half + kqh
                            for g, nr in ((0, nrA), (1, nrB)):
                                nc.tensor.matmul(
                                    out=ps[:, 2 * kqh + g, 0:NST],
                                    lhsT=wbt[0:nr * KS,
                                             kq * 128:(kq + 1) * 128],
                                    rhs=pt2[0:nr * KS, g, :],
                                    start=True, stop=True)
                        # bank-split evacuation: DVE banks 0-1, ACT 2-3
                        nc.vector.tensor_copy(
                            out=chunk[:, 4 * half: 4 * half + 2, :],
                            in_=ps[:, 0:2, 0:NST])
                        nc.scalar.copy(
                            out=chunk[:, 4 * half + 2: 4 * half + 4, :],
                            in_=ps[:, 2:4, 0:NST])
                    nc.sync.dma_start(out=out_d[q, st], in_=chunk[:])
    nc.compile()
    return nc


def _get_nc():
    if "nc" not in _NC_CACHE:
        _NC_CACHE["nc"] = _build_nc()
    return _NC_CACHE["nc"]


def _prep_x(x: np.ndarray) -> np.ndarray:
    """[B, H, W] f32 -> per-core flat [2*XIMG] bf16 x NCORES, images
    interleaved per pair, rows padded to SEGW (=464) elems."""
    xb = x.astype(ml_dtypes.bfloat16)
    xp = xb.reshape(B // 2, 2, H, W).transpose(0, 2, 1, 3)
    buf = np.zeros((B // 2, H, SEGW), dtype=ml_dtypes.bfloat16)
    buf[:, :, :2 * W] = xp.reshape(B // 2, H, 2 * W)
    return buf.reshape(NCORES, 2 * XIMG)


def _untangle(arr: np.ndarray) -> np.ndarray:
    """Per-core raw chunk dump [2, NSTEP, 128, 8*NST] bf16 ->
    [4, KCH, HO, WO] f32."""
    # [q, st, s*16+kc, ((kq*2 + g)*2 + img)*224 + j]
    a = arr.reshape(2, NSTEP, 8, 16, 4, 2, 2, W)  # q st s kc kq g img j
    a = a.transpose(0, 6, 4, 3, 1, 5, 2, 7)       # q img kq kc st g s j
    a = a.reshape(BLOC, KCH, H, W)
    return a[:, :, :HO, :WO].astype(np.float32)


def kernel(x: np.ndarray, kernels: np.ndarray) -> np.ndarray:
    from concourse.bass_utils import run_bass_kernel_spmd

    x = np.asarray(x, dtype=np.float32)
    kernels = np.asarray(kernels, dtype=np.float32)
    xp = _prep_x(x)  # [NCORES, 2*XIMG]
    wb = make_weight_band(kernels)
    nc = _get_nc()
    in_maps = [
        {"x": xp[c], "wband": wb}
        for c in range(NCORES)
    ]
    res = run_bass_kernel_spmd(nc, in_maps, core_ids=list(range(NCORES)))
    return np.ascontiguousarray(np.concatenate(
        [_untangle(res.results[c]["out"]) for c in range(NCORES)], axis=0))
